# revision 24
# baseline (speedup 1.0000x reference)
"""Trainium2 Bass kernel for nn_CategoryAdder (embedding lookup + masked add).

Computation: out[b,s,:] = inputs[b,s,:] + emb where
  emb = table[categories[b,s]] masked to zero when categories[b,s]==0 or
  s == mask_positions[b].

Host-side preprocessing folds both masks into the data:
  - categories[b, mask_positions[b]] = 0
  - table row 0 zeroed (on a copy)
so the device computes exactly: out = inputs + table0[categories].

Final design (~152 us vs the 347 us fp32 baseline): the baseline was
DMA-bus-bound (100.9 MB/core over the 16-engine ~370 GB/s bus). Everything
on device is bf16 now (rel tolerance 2e-2 >> bf16's ~2.5e-3), halving bus
bytes to 50.4 MB/core (~135 us floor): x and out stream as bf16 and the
SWDGE dma_gather pulls 1 KB bf16 table rows. Q7 descriptor generation
(~8.6 ns/idx, 141 us serial) is spread round-robin across 4 SWDGE queues —
each queue's desc-gen runs on its own pair of the 8 Q7 cores — which takes
it off the critical path (all desc-gen done by ~40 us). Loads ride the sync
engine's HWDGE queue and stores the scalar engine's, so the two streams
never head-of-line block each other.

Sharding: data-parallel over batch across 8 NeuronCores (8 batches per core,
16384 tokens/core). Table replicated.
"""

import numpy as np
import ml_dtypes

import concourse.mybir as mybir
from concourse import bacc, tile
from concourse.bass_utils import run_bass_kernel_spmd

BF16 = ml_dtypes.bfloat16


def _ensure_axon_ntff_hook_module():
    """run_bass_kernel_spmd(trace=True) under axon imports antenv.axon_hooks,
    which this image lacks — install a fallback shim (backed by the boot
    module's ctypes hook when available) so a BASS_TRACE=1 environment does
    not crash the kernel. No-op when the real module exists."""
    try:
        import antenv.axon_hooks  # noqa: F401
        return
    except ImportError:
        pass
    import sys
    import types

    hook = None
    try:
        import trn_agent_boot.trn_boot as _tb

        hook = _tb._ntff_profile_via_ctypes("/opt/axon/libaxon_pjrt.so")
    except Exception:
        hook = None  # get_..._hook() -> None makes bass_utils skip tracing
    mod = types.ModuleType("antenv.axon_hooks")
    mod.get_axon_ntff_profile_hook = lambda: hook
    mod.set_axon_ntff_profile_hook = lambda h: None
    sys.modules["antenv.axon_hooks"] = mod


_ensure_axon_ntff_hook_module()

B, S, D = 64, 2048, 512
N_CAT = 5000
N_CORES = 8
B_PER = B // N_CORES          # 8 batches per core
NTOK = B_PER * S              # 16384 tokens per core
IDX_COLS = NTOK // 16         # columns of the wrapped int16 index tensor
N_QUEUES = 4

# All gathers are issued up-front (emb pools cover every tile) so 4-queue
# desc-gen runs flat out and gather SDMA traffic is fully decoupled from the
# load/add/store pipeline. Small head tiles let the first adds (and the
# load-buffer recycling they gate) start as soon as the Q7 ucode library
# finishes loading (~23 us); small tail tiles shorten the final
# add+store chain after the last gather packets land.
TILES = [256, 256, 512] + [1024] * 14 + [512, 256, 256]
assert sum(TILES) == NTOK


def _build_nc():
    nc = bacc.Bacc(
        "TRN2", target_bir_lowering=False, debug=False, num_swdge_queues=N_QUEUES
    )
    x = nc.dram_tensor("x", [NTOK, D], mybir.dt.bfloat16, kind="ExternalInput")
    tbl = nc.dram_tensor("tbl", [N_CAT, D], mybir.dt.bfloat16, kind="ExternalInput")
    idx = nc.dram_tensor("idx", [128, IDX_COLS], mybir.dt.int16, kind="ExternalInput")
    out = nc.dram_tensor("out", [NTOK, D], mybir.dt.bfloat16, kind="ExternalOutput")

    with tile.TileContext(nc) as tc:
        n_small = sum(1 for T in TILES if T < 1024)
        n_big = len(TILES) - n_small
        with (
            tc.tile_pool(name="idxp", bufs=1) as idxp,
            tc.tile_pool(name="inp", bufs=8) as inp,
            tc.tile_pool(name="embs", bufs=n_small) as embs,
            tc.tile_pool(name="embb", bufs=n_big) as embb,
        ):
            idx_sb = idxp.tile([128, IDX_COLS], mybir.dt.int16)
            nc.sync.dma_start(out=idx_sb[:], in_=idx[:, :])
            # Gather issue order: head smalls, then TAIL smalls, then bigs.
            # Queue packets drain roughly in issue order, so this lands the
            # tail tiles' embeddings early — the final adds then only wait
            # on the last big tile instead of the very end of the slow
            # 1 KB-packet gather drain.
            cols = []
            col = 0
            for T in TILES:
                cols.append(col)
                col += T // 16
            n = len(TILES)
            order = [0, 1, 2, n - 3, n - 2, n - 1] + list(range(3, n - 3))
            emb_tiles = [None] * n
            for qpos, i in enumerate(order):
                T = TILES[i]
                C = T // 128
                pool = embb if T == 1024 else embs
                emb_t = pool.tile([128, C * D], mybir.dt.bfloat16, tag="emb")
                nc.gpsimd.dma_gather(
                    emb_t[:].rearrange("p (c e) -> p c e", e=D),
                    tbl[:, :],
                    idx_sb[:, cols[i] : cols[i] + T // 16],
                    T,
                    T,
                    D,
                    single_packet=False,
                    queue_num=qpos % N_QUEUES,
                )
                emb_tiles[i] = emb_t
            t0 = 0
            for i, T in enumerate(TILES):
                C = T // 128
                in_t = inp.tile([128, C * D], mybir.dt.bfloat16, tag="in")
                nc.sync.dma_start(
                    out=in_t[:],
                    in_=x[t0 : t0 + T].rearrange("(p c) e -> p (c e)", p=128),
                )
                nc.vector.tensor_add(out=in_t[:], in0=in_t[:], in1=emb_tiles[i][:])
                # stores share the sync engine's HWDGE queue with loads:
                # one 8 KB-desc hw queue instead of two raises the 1 KB
                # gather queues' per-descriptor arbitration share
                nc.sync.dma_start(
                    out=out[t0 : t0 + T].rearrange("(p c) e -> p (c e)", p=128),
                    in_=in_t[:],
                )
                t0 += T
    nc.compile()
    return nc


def _prep_idx(cat_shard: np.ndarray) -> np.ndarray:
    """cat_shard: (NTOK,) int -> wrapped int16 index tensor [128, IDX_COLS].

    dma_gather writes gather-slot i to SBUF (partition i%128, column i//128);
    our tiles place token t at (partition t//C, column t%C), so slot i holds
    the category of token (i%128)*C + i//128. Indices are then wrapped 16-way
    (idxs[p, s] = slot s*16+p) and replicated across the 8 groups of 16
    partitions so any SWDGE queue's core pair reads the same list.
    """
    blocks = []
    t0 = 0
    for T in TILES:
        C = T // 128
        slot_to_token = (np.arange(T) % 128) * C + (np.arange(T) // 128)
        vals = cat_shard[t0 : t0 + T][slot_to_token]
        blocks.append(np.tile(vals.reshape(T // 16, 16).T, (8, 1)))
        t0 += T
    return np.ascontiguousarray(np.concatenate(blocks, axis=1).astype(np.int16))


RUN_KWARGS = {}  # test harness can set e.g. {"trace": True}
LAST_RESULTS = None
_NC = None


def _get_nc():
    global _NC
    if _NC is None:
        _NC = _build_nc()
    return _NC


def kernel(inputs, categories, mask_positions, table):
    global LAST_RESULTS
    inputs = np.asarray(inputs, dtype=np.float32)
    categories = np.asarray(categories).astype(np.int64)
    mask_positions = np.asarray(mask_positions).astype(np.int64)
    table = np.asarray(table, dtype=np.float32)

    # Fold both masks into the data.
    cat = categories.copy()
    cat[np.arange(B), mask_positions[:, 0]] = 0
    tbl0 = table.astype(BF16)
    tbl0[0] = 0.0

    x16 = inputs.astype(BF16)  # one fp32->bf16 pass over the full input

    nc = _get_nc()

    in_maps = []
    for c in range(N_CORES):
        x_shard = np.ascontiguousarray(
            x16[c * B_PER : (c + 1) * B_PER].reshape(NTOK, D)
        )
        cat_shard = cat[c * B_PER : (c + 1) * B_PER].reshape(NTOK)
        in_maps.append({"x": x_shard, "tbl": tbl0, "idx": _prep_idx(cat_shard)})

    res = run_bass_kernel_spmd(
        nc, in_maps, core_ids=list(range(N_CORES)), **RUN_KWARGS
    )
    LAST_RESULTS = res
    out = np.concatenate(
        [
            np.asarray(r["out"]).astype(np.float32).reshape(B_PER, S, D)
            for r in res.results
        ],
        axis=0,
    )
    return out


# revision 26
# speedup vs baseline: 1.1581x; 1.1581x over previous
"""Trainium2 Bass kernel for nn_CategoryAdder (embedding lookup + masked add).

Computation: out[b,s,:] = inputs[b,s,:] + emb where
  emb = table[categories[b,s]] masked to zero when categories[b,s]==0 or
  s == mask_positions[b].

Host-side preprocessing folds both masks into the data:
  - categories[b, mask_positions[b]] = 0
  - table row 0 zeroed (on a copy)
so the device computes exactly: out = inputs + table0[categories].

Final design (~152 us vs the 347 us fp32 baseline): the baseline was
DMA-bus-bound (100.9 MB/core over the 16-engine ~370 GB/s bus). Everything
on device is bf16 now (rel tolerance 2e-2 >> bf16's ~2.5e-3), halving bus
bytes to 50.4 MB/core (~135 us floor): x and out stream as bf16 and the
SWDGE dma_gather pulls 1 KB bf16 table rows. Q7 descriptor generation
(~8.6 ns/idx, 141 us serial) is spread round-robin across 4 SWDGE queues —
each queue's desc-gen runs on its own pair of the 8 Q7 cores — which takes
it off the critical path (all desc-gen done by ~40 us). Loads ride the sync
engine's HWDGE queue and stores the scalar engine's, so the two streams
never head-of-line block each other.

Sharding: data-parallel over batch across 8 NeuronCores (8 batches per core,
16384 tokens/core). Table replicated.
"""

import numpy as np
import ml_dtypes

import concourse.mybir as mybir
from concourse import bacc, tile
from concourse.bass_utils import run_bass_kernel_spmd

BF16 = ml_dtypes.bfloat16


def _ensure_axon_ntff_hook_module():
    """run_bass_kernel_spmd(trace=True) under axon imports antenv.axon_hooks,
    which this image lacks — install a fallback shim (backed by the boot
    module's ctypes hook when available) so a BASS_TRACE=1 environment does
    not crash the kernel. No-op when the real module exists."""
    try:
        import antenv.axon_hooks  # noqa: F401
        return
    except ImportError:
        pass
    import sys
    import types

    hook = None
    try:
        import trn_agent_boot.trn_boot as _tb

        hook = _tb._ntff_profile_via_ctypes("/opt/axon/libaxon_pjrt.so")
    except Exception:
        hook = None  # get_..._hook() -> None makes bass_utils skip tracing
    mod = types.ModuleType("antenv.axon_hooks")
    mod.get_axon_ntff_profile_hook = lambda: hook
    mod.set_axon_ntff_profile_hook = lambda h: None
    sys.modules["antenv.axon_hooks"] = mod


_ensure_axon_ntff_hook_module()

B, S, D = 64, 2048, 512
N_CAT = 5000
N_CORES = 8
B_PER = B // N_CORES          # 8 batches per core
NTOK = B_PER * S              # 16384 tokens per core
IDX_COLS = NTOK // 16         # columns of the wrapped int16 index tensor
N_QUEUES = 4

# All gathers are issued up-front (emb pools cover every tile) so 4-queue
# desc-gen runs flat out and gather SDMA traffic is fully decoupled from the
# load/add/store pipeline. Small head tiles let the first adds (and the
# load-buffer recycling they gate) start as soon as the Q7 ucode library
# finishes loading (~23 us); small tail tiles shorten the final
# add+store chain after the last gather packets land.
TILES = [256, 256, 512] + [1024] * 14 + [512, 256, 256]
assert sum(TILES) == NTOK


def _build_nc():
    nc = bacc.Bacc(
        "TRN2", target_bir_lowering=False, debug=False, num_swdge_queues=N_QUEUES
    )
    x = nc.dram_tensor("x", [NTOK, D], mybir.dt.bfloat16, kind="ExternalInput")
    tbl = nc.dram_tensor("tbl", [N_CAT, D], mybir.dt.bfloat16, kind="ExternalInput")
    idx = nc.dram_tensor("idx", [128, IDX_COLS], mybir.dt.int16, kind="ExternalInput")
    out = nc.dram_tensor("out", [NTOK, D], mybir.dt.bfloat16, kind="ExternalOutput")

    with tile.TileContext(nc) as tc:
        n_small = sum(1 for T in TILES if T < 1024)
        n_big = len(TILES) - n_small
        with (
            tc.tile_pool(name="idxp", bufs=1) as idxp,
            tc.tile_pool(name="inp", bufs=8) as inp,
            tc.tile_pool(name="embs", bufs=n_small) as embs,
            tc.tile_pool(name="embb", bufs=n_big) as embb,
        ):
            idx_sb = idxp.tile([128, IDX_COLS], mybir.dt.int16)
            nc.sync.dma_start(out=idx_sb[:], in_=idx[:, :])
            # Gather issue order: head smalls, then TAIL smalls, then bigs.
            # Queue packets drain roughly in issue order, so this lands the
            # tail tiles' embeddings early — the final adds then only wait
            # on the last big tile instead of the very end of the slow
            # 1 KB-packet gather drain.
            cols = []
            col = 0
            for T in TILES:
                cols.append(col)
                col += T // 16
            n = len(TILES)
            order = [0, 1, 2, n - 3, n - 2, n - 1] + list(range(3, n - 3))
            emb_tiles = [None] * n
            for qpos, i in enumerate(order):
                T = TILES[i]
                C = T // 128
                pool = embb if T == 1024 else embs
                emb_t = pool.tile([128, C * D], mybir.dt.bfloat16, tag="emb")
                nc.gpsimd.dma_gather(
                    emb_t[:].rearrange("p (c e) -> p c e", e=D),
                    tbl[:, :],
                    idx_sb[:, cols[i] : cols[i] + T // 16],
                    T,
                    T,
                    D,
                    single_packet=False,
                    queue_num=qpos % N_QUEUES,
                )
                emb_tiles[i] = emb_t
            t0 = 0
            for i, T in enumerate(TILES):
                C = T // 128
                in_t = inp.tile([128, C * D], mybir.dt.bfloat16, tag="in")
                # Big tiles split loads/stores into two 4 KB-per-partition
                # descriptor halves: the DMA engines round-robin queues per
                # descriptor, so halving the 8 KB load/store descs doubles
                # the bus share of the four 1 KB-packet gather queues (whose
                # slow drain gates the final adds) while 4 KB stays at the
                # descriptor-efficiency saturation threshold.
                halves = 2 if T == 1024 else 1
                xv = x[t0 : t0 + T].rearrange("(p c) e -> p (c e)", p=128)
                ov = out[t0 : t0 + T].rearrange("(p c) e -> p (c e)", p=128)
                h = C * D // halves
                for k in range(halves):
                    nc.sync.dma_start(
                        out=in_t[:, k * h : (k + 1) * h],
                        in_=xv[:, k * h : (k + 1) * h],
                    )
                nc.vector.tensor_add(out=in_t[:], in0=in_t[:], in1=emb_tiles[i][:])
                # stores ride the scalar engine's HWDGE queue so the late
                # store stream never head-of-line blocks the early loads
                for k in range(halves):
                    nc.scalar.dma_start(
                        out=ov[:, k * h : (k + 1) * h],
                        in_=in_t[:, k * h : (k + 1) * h],
                    )
                t0 += T
    nc.compile()
    return nc


def _prep_idx(cat_shard: np.ndarray) -> np.ndarray:
    """cat_shard: (NTOK,) int -> wrapped int16 index tensor [128, IDX_COLS].

    dma_gather writes gather-slot i to SBUF (partition i%128, column i//128);
    our tiles place token t at (partition t//C, column t%C), so slot i holds
    the category of token (i%128)*C + i//128. Indices are then wrapped 16-way
    (idxs[p, s] = slot s*16+p) and replicated across the 8 groups of 16
    partitions so any SWDGE queue's core pair reads the same list.
    """
    blocks = []
    t0 = 0
    for T in TILES:
        C = T // 128
        slot_to_token = (np.arange(T) % 128) * C + (np.arange(T) // 128)
        vals = cat_shard[t0 : t0 + T][slot_to_token]
        blocks.append(np.tile(vals.reshape(T // 16, 16).T, (8, 1)))
        t0 += T
    return np.ascontiguousarray(np.concatenate(blocks, axis=1).astype(np.int16))


RUN_KWARGS = {}  # test harness can set e.g. {"trace": True}
LAST_RESULTS = None
_NC = None


def _get_nc():
    global _NC
    if _NC is None:
        _NC = _build_nc()
    return _NC


def kernel(inputs, categories, mask_positions, table):
    global LAST_RESULTS
    inputs = np.asarray(inputs, dtype=np.float32)
    categories = np.asarray(categories).astype(np.int64)
    mask_positions = np.asarray(mask_positions).astype(np.int64)
    table = np.asarray(table, dtype=np.float32)

    # Fold both masks into the data.
    cat = categories.copy()
    cat[np.arange(B), mask_positions[:, 0]] = 0
    tbl0 = table.astype(BF16)
    tbl0[0] = 0.0

    x16 = inputs.astype(BF16)  # one fp32->bf16 pass over the full input

    nc = _get_nc()

    in_maps = []
    for c in range(N_CORES):
        x_shard = np.ascontiguousarray(
            x16[c * B_PER : (c + 1) * B_PER].reshape(NTOK, D)
        )
        cat_shard = cat[c * B_PER : (c + 1) * B_PER].reshape(NTOK)
        in_maps.append({"x": x_shard, "tbl": tbl0, "idx": _prep_idx(cat_shard)})

    res = run_bass_kernel_spmd(
        nc, in_maps, core_ids=list(range(N_CORES)), **RUN_KWARGS
    )
    LAST_RESULTS = res
    out = np.concatenate(
        [
            np.asarray(r["out"]).astype(np.float32).reshape(B_PER, S, D)
            for r in res.results
        ],
        axis=0,
    )
    return out
